# revision 8
# baseline (speedup 1.0000x reference)
"""Trainium2 Bass/Tile kernel for nn_FB_FMM (sparse_attention).

Computation (per batch element b, with N = H*W = 4096 tokens, C=256, D=32):
  1. Self-attention:  sa_out = attn(conv(x,sa_wq), conv(x,sa_wk), conv(x,sa_wv))
     x' = sa_gamma * sa_out + x
  2. Masked cross-attention (FB_FMM):
     ff = mask * x'; fb = (1-mask) * x'
     sw_bg = attn(conv(ff,wq), conv(fb,wk), conv(fb,wv))
     out = x' + gamma * ff * (std(sw_bg)/std(ff))    [per-channel std, ddof=1]

Sharding: 8 cores = 2 batch groups x 4-way query-row sharding (1024 rows each).
Each core computes its row-chunk of both attention layers; K/V sides are
computed redundantly per core. Cross-core communication:
  - AllGather of x' chunks within each 4-core batch group (layer-2 K/V need
    the full x'), split into two 512-row phases; layer-2 attention processes
    phase-0 key tiles first so phase 1 hides under compute.
  - AllReduce of per-channel [sum, sumsq] stats for the FMM std ratio.

Layouts: feature maps are channel-major (C on partitions). Scores are computed
transposed (S^T: keys j on partitions, queries i free; logits are small enough
that exp needs no max-subtraction pass). The AV matmul keeps V^T slices
stationary with E^T moving, producing O in natural (c x i) layout. The softmax
denominator is NOT a matmul over E: E tiles are binary-tree-summed on the
Vector engine (bf16) and one ones-matmul per i-chunk reduces the partition
axis; the denominator row is replicated across partitions with a K=1 ones
matmul and reciprocated off the PE critical path. Conv biases: K-side biases
drop out exactly (they add a per-query constant to logits, cancelled by the
row softmax); Q biases are applied on the Scalar engine (Identity+bias).
V-conv PSUM->SBUF copies run on the Scalar engine, K copies on Vector, and
the feature_b masking muls on GpSimd, balancing the three element-wise
engines. V-conv biases fold out mathematically (layer-1 into the residual;
layer-2 via shift invariance of variance). PSUM: 2 conv/den banks + 2 S^T
banks + 4 AV accumulator banks, so consecutive i-chunks overlap without the
PE waiting on epilogues.
"""

import numpy as np

P = 128
B, C, HH, WW = 2, 256, 64, 64
N = HH * WW            # 4096 tokens
D = 32                 # q/k channels
NCORES = 8
RSH = 4                # row shards per batch group
R = N // RSH           # 1024 query rows per core
NT = N // P            # 32 key tiles
IC = 512               # query i-chunk (one PSUM bank of fp32)
EPS = 1e-5
F32 = np.float32

_CACHE = {}

# layer-2 key-tile order: AllGather phase-0 tiles (cols [r*1024, r*1024+512))
# first, phase-1 tiles after
TILES_PH0 = [t for t in range(NT) if (t % 8) < 4]
TILES_PH1 = [t for t in range(NT) if (t % 8) >= 4]


def _build_bass():
    """Build the Bass/Tile program (single SPMD NEFF for all 8 cores)."""
    import concourse.bass as bass
    from concourse import bacc, mybir, tile

    f32 = mybir.dt.float32
    f32r = mybir.dt.float32r
    bf16 = mybir.dt.bfloat16
    AX = mybir.AxisListType
    OP = mybir.AluOpType
    AF = mybir.ActivationFunctionType

    nc = bacc.Bacc(
        "TRN2", target_bir_lowering=False, debug=False, num_devices=NCORES
    )

    # ---------------- I/O ----------------
    xf_d = nc.dram_tensor("xf", [C, N], bf16, kind="ExternalInput")
    xc_d = nc.dram_tensor("xc", [C, R], f32r, kind="ExternalInput")
    rm_d = nc.dram_tensor("rm", [1, N], bf16, kind="ExternalInput")
    mcrow_d = nc.dram_tensor("mcrow", [1, R], f32, kind="ExternalInput")
    wqT1_d = nc.dram_tensor("wqT1", [C, D], f32r, kind="ExternalInput")
    wkT1_d = nc.dram_tensor("wkT1", [C, D], bf16, kind="ExternalInput")
    wvT1_d = nc.dram_tensor("wvT1", [C, C], bf16, kind="ExternalInput")
    wqT2_d = nc.dram_tensor("wqT2", [C, D], f32r, kind="ExternalInput")
    wkT2_d = nc.dram_tensor("wkT2", [C, D], bf16, kind="ExternalInput")
    wvT2_d = nc.dram_tensor("wvT2", [C, C], bf16, kind="ExternalInput")
    # consts columns: 0 sa_gamma, 1 gamma, 2/3 sa_gamma*sa_bv halves,
    # 4 gamma^2, 6 sa_bq, 8 bq (cols 6/8 live on partitions 0..31)
    consts_d = nc.dram_tensor("consts", [P, 10], f32, kind="ExternalInput")
    out_d = nc.dram_tensor("outc", [C, R], f32, kind="ExternalOutput")

    groups = [[0, 1, 2, 3], [4, 5, 6, 7]]

    with tile.TileContext(nc) as tc:
        from contextlib import ExitStack

        ctx = ExitStack()
        with ctx:
            big = ctx.enter_context(tc.tile_pool(name="big", bufs=1))
            epool = ctx.enter_context(tc.tile_pool(name="epool", bufs=4))
            tpool = ctx.enter_context(tc.tile_pool(name="tpool", bufs=7))
            onpool = ctx.enter_context(tc.tile_pool(name="onpool", bufs=3))
            sqpool = ctx.enter_context(tc.tile_pool(name="sqpool", bufs=2))
            fbpool = ctx.enter_context(tc.tile_pool(name="fbpool", bufs=4))
            rcpool = ctx.enter_context(tc.tile_pool(name="rcpool", bufs=4))
            finpool = ctx.enter_context(tc.tile_pool(name="finpool", bufs=2))
            misc = ctx.enter_context(tc.tile_pool(name="misc", bufs=1))
            psA = ctx.enter_context(
                tc.tile_pool(name="psA", bufs=2, space="PSUM")
            )
            psS = ctx.enter_context(
                tc.tile_pool(name="psS", bufs=2, space="PSUM")
            )
            psO = ctx.enter_context(
                tc.tile_pool(name="psO", bufs=4, space="PSUM")
            )
            dram = ctx.enter_context(
                tc.tile_pool(name="dram", bufs=1, space="DRAM")
            )

            # ------------- persistent SBUF tiles -------------
            xc_sb = big.tile([P, 2, R], f32r, tag="xc", name="xc_sb")
            rmask_sb = big.tile([P, N], bf16, tag="rmask", name="rmask_sb")
            maskc_sb = big.tile([P, R], f32, tag="maskc", name="maskc_sb")
            xp_sb = big.tile([P, 2, R], f32, tag="xp", name="xp_sb")
            xp16_sb = big.tile([P, 2, R], bf16, tag="xp16", name="xp16_sb")
            ff_sb = big.tile([P, 2, R], f32r, tag="ff", name="ff_sb")
            wqT1_sb = big.tile([P, 2, D], f32r, tag="wqT1", name="wqT1_sb")
            wkT1_sb = big.tile([P, 2, D], bf16, tag="wkT1", name="wkT1_sb")
            wvT1_sb = big.tile([P, 2, C], bf16, tag="wvT1", name="wvT1_sb")
            wqT2_sb = big.tile([P, 2, D], f32r, tag="wqT2", name="wqT2_sb")
            wkT2_sb = big.tile([P, 2, D], bf16, tag="wkT2", name="wkT2_sb")
            wvT2_sb = big.tile([P, 2, C], bf16, tag="wvT2", name="wvT2_sb")
            consts_sb = big.tile([P, 10], f32, tag="consts", name="consts_sb")
            # ones column (bf16) for the denominator matmul; ones row (f32r)
            # for the K=1 replication matmul
            onesc_sb = big.tile([P, 1], bf16, tag="onesc", name="onesc_sb")
            onesr_sb = big.tile([1, P], f32r, tag="onesr", name="onesr_sb")
            # stats: cols 0-3 = sums (ff0, ff1, sw0, sw1), 4-7 = sumsqs
            stats_sb = misc.tile([P, 8], f32, tag="stats", name="stats_sb")
            xf_sb = big.tile([P, 2, N], bf16, tag="xbig", name="xf_sb")

            # ---- input DMAs, ordered for earliest PE start ----
            # sync queue: Q-conv deps first (consts, wqT1, xc)
            nc.sync.dma_start(out=consts_sb[:], in_=consts_d[:])
            for k in range(2):
                cs = slice(k * P, (k + 1) * P)
                nc.sync.dma_start(out=wqT1_sb[:, k, :], in_=wqT1_d[cs, :])
            for k in range(2):
                cs = slice(k * P, (k + 1) * P)
                nc.sync.dma_start(out=xc_sb[:, k, :], in_=xc_d[cs, :])
            # gpsimd queue: the two big xf halves (1 MB each)
            for k in range(2):
                nc.gpsimd.dma_start(
                    out=xf_sb[:, k, :], in_=xf_d[k * P : (k + 1) * P, :]
                )
            # scalar queue: K/V weights, then layer-2 weights + masks
            for k in range(2):
                cs = slice(k * P, (k + 1) * P)
                nc.scalar.dma_start(out=wkT1_sb[:, k, :], in_=wkT1_d[cs, :])
                nc.scalar.dma_start(out=wvT1_sb[:, k, :], in_=wvT1_d[cs, :])
            for k in range(2):
                cs = slice(k * P, (k + 1) * P)
                nc.scalar.dma_start(out=wqT2_sb[:, k, :], in_=wqT2_d[cs, :])
                nc.scalar.dma_start(out=wkT2_sb[:, k, :], in_=wkT2_d[cs, :])
                nc.scalar.dma_start(out=wvT2_sb[:, k, :], in_=wvT2_d[cs, :])
            nc.scalar.dma_start(
                out=rmask_sb[:], in_=rm_d[0, :].partition_broadcast(P)
            )
            nc.scalar.dma_start(
                out=maskc_sb[:], in_=mcrow_d[0, :].partition_broadcast(P)
            )
            nc.vector.memset(onesc_sb[:], 1.0)
            nc.vector.memset(onesr_sb[:].bitcast(f32), 1.0)

            # ---- collective warmup: tiny AllGather overlapping the head ----
            warm_in = dram.tile([P, 1], f32, tag="warm_in", name="warm_in")
            warm_out = dram.tile(
                [RSH, P, 1], f32, tag="warm_out", name="warm_out"
            )
            nc.sync.dma_start(out=warm_in[:], in_=consts_sb[:, 0:1])
            nc.gpsimd.collective_compute(
                "AllGather",
                OP.bypass,
                replica_groups=groups,
                ins=[warm_in[:].opt()],
                outs=[warm_out[:].opt()],
            )

            def conv_qk(wT_sb, bias_col, src_of, width, out_sb):
                """out (D x width) = wT.T @ src (+ bias via ACT Identity).
                bias_col=None -> plain DVE copy (K convs: bias cancels in
                row-softmax)."""
                for jc in range(width // IC):
                    js = slice(jc * IC, (jc + 1) * IC)
                    ps = psA.tile([D, IC], f32, tag="a", name="qk_ps")
                    nc.tensor.matmul(
                        ps[:], wT_sb[:, 0, :], src_of(0, js),
                        start=True, stop=False,
                    )
                    nc.tensor.matmul(
                        ps[:], wT_sb[:, 1, :], src_of(1, js),
                        start=False, stop=True,
                    )
                    if bias_col is None:
                        nc.vector.tensor_copy(out_sb[:, js], ps[:])
                    else:
                        nc.scalar.activation(
                            out_sb[:, js], ps[:], AF.Identity,
                            bias=consts_sb[0:D, bias_col : bias_col + 1],
                        )

            def conv_vT(wvT_sb, src_of, v_sb, t):
                """v_sb[:, t, :] = (src^T @ wvT) for key tile t (j on
                partitions, channels free)."""
                ts_ = slice(t * P, (t + 1) * P)
                ps = psA.tile([P, C], f32, tag="a", name="v_ps")
                nc.tensor.matmul(
                    ps[:], src_of(0, ts_), wvT_sb[:, 0, :],
                    start=True, stop=False,
                )
                nc.tensor.matmul(
                    ps[:], src_of(1, ts_), wvT_sb[:, 1, :],
                    start=False, stop=True,
                )
                nc.scalar.copy(v_sb[:, t, :], ps[:])

            def attention(q_sb, k_sb, v_sb, epilogue, chunks, orders=None,
                          hooks=None):
                """Row-chunk attention.  Per i-chunk: S^T = K-tile^T Q
                (j on partitions), E = exp(S^T), then O(c,i) accumulates
                with V^T slices stationary and E moving.  The softmax
                denominator comes from a DVE binary-tree sum of the E tiles
                (bf16) + one ones-matmul; the replicated reciprocal is
                computed off the PE critical path.  chunks gives the
                i-chunk widths; orders[ich] the key-tile processing order;
                hooks[(ich, pos)] emits extra work (e.g. convs gated on an
                AllGather) before the s_exp of loop position pos."""
                LOOKAHEAD = 2
                base = 0
                for ich, ic in enumerate(chunks):
                    order = (orders[ich] if orders else None) or list(
                        range(NT)
                    )
                    hk = hooks or {}
                    is_ = slice(base, base + ic)
                    accs = [
                        psO.tile([P, ic], f32, tag="o", name="acc")
                        for _ in range(2)
                    ]
                    es = {}
                    # binary-counter partial sums of E tiles (bf16)
                    partials = [None] * 6

                    def s_exp(pos):
                        t = order[pos]
                        sps = psS.tile([P, ic], f32, tag="s", name="s_ps")
                        nc.tensor.matmul(
                            sps[:],
                            k_sb[:, t * P : (t + 1) * P],
                            q_sb[:, is_],
                            start=True, stop=True,
                        )
                        e_sb = epool.tile([P, ic], bf16, tag="e", name="e_sb")
                        nc.scalar.activation(e_sb[:], sps[:], AF.Exp)
                        es[pos] = e_sb
                        # fold into the tree (DVE, bf16)
                        carry, lev = e_sb, 0
                        while partials[lev] is not None:
                            nxt = tpool.tile(
                                [P, ic], bf16, tag="tp", name="tp"
                            )
                            nc.vector.tensor_add(
                                nxt[:], partials[lev][:], carry[:]
                            )
                            partials[lev] = None
                            carry, lev = nxt, lev + 1
                        partials[lev] = carry

                    for pos in range(LOOKAHEAD):
                        if (ich, pos) in hk:
                            hk[(ich, pos)]()
                        s_exp(pos)
                    for pos in range(NT):
                        if pos + LOOKAHEAD < NT:
                            if (ich, pos + LOOKAHEAD) in hk:
                                hk[(ich, pos + LOOKAHEAD)]()
                            s_exp(pos + LOOKAHEAD)
                        t = order[pos]
                        e_sb = es.pop(pos)
                        for ct in range(2):
                            nc.tensor.matmul(
                                accs[ct][:],
                                v_sb[:, t, ct * P : (ct + 1) * P],
                                e_sb[:],
                                start=(pos == 0), stop=(pos == NT - 1),
                            )
                    esum = partials[5]  # 32 tiles -> exactly level 5
                    den = psA.tile([1, ic], f32, tag="a", name="den")
                    nc.tensor.matmul(
                        den[:], onesc_sb[:], esum[:], start=True, stop=True
                    )
                    denr = rcpool.tile([1, ic], f32r, tag="rc", name="denr")
                    nc.vector.tensor_copy(denr[:], den[:])
                    drep_ps = psA.tile([P, ic], f32, tag="a", name="drep_ps")
                    nc.tensor.matmul(
                        drep_ps[:], onesr_sb[:], denr[:],
                        start=True, stop=True,
                    )
                    rrep = onpool.tile([P, ic], f32, tag="rr", name="rrep")
                    nc.vector.reciprocal(rrep[:], drep_ps[:])
                    epilogue(ich, is_, ic, accs, rrep)
                    base += ic

            # ================= Layer 1: self-attention =================
            q1_sb = big.tile([D, R], bf16, tag="q", name="q1_sb")
            k1_sb = big.tile([D, N], bf16, tag="k", name="k1_sb")
            v1_sb = big.tile([P, NT, C], bf16, tag="v", name="v1_sb")

            conv_qk(wqT1_sb, 6, lambda k, js: xc_sb[:, k, js], R, q1_sb)
            conv_qk(wkT1_sb, None, lambda k, js: xf_sb[:, k, js], N, k1_sb)
            for t in range(NT):
                conv_vT(wvT1_sb, lambda k, ts_: xf_sb[:, k, ts_], v1_sb, t)

            def epilogue1(ich, io, ic, accs, rrep):
                for ct in range(2):
                    # x' = sa_gamma * (O/den) + sa_gamma*bv + x, fused as
                    # ((O * sa_gamma) * rrep), then ((t + sgb) + x)
                    nc.vector.scalar_tensor_tensor(
                        xp_sb[:, ct, io], accs[ct][:],
                        consts_sb[:, 0:1], rrep[:],
                        op0=OP.mult, op1=OP.mult,
                    )
                    nc.vector.scalar_tensor_tensor(
                        xp_sb[:, ct, io], xp_sb[:, ct, io],
                        consts_sb[:, 2 + ct : 3 + ct],
                        xc_sb[:, ct, io].bitcast(f32),
                        op0=OP.add, op1=OP.add,
                    )
                    nc.vector.tensor_copy(
                        xp16_sb[:, ct, io], xp_sb[:, ct, io]
                    )

            attention(q1_sb, k1_sb, v1_sb, epilogue1, [IC, IC])

            # ====== AllGather x' within each batch group (2 phases) ======
            # Phase h gathers x' columns [h*512, (h+1)*512) of every rank;
            # phase 0 overlaps the second layer-1 attention i-chunk.
            xpf_sb = big.tile([P, 2, N], bf16, tag="xpf", name="xpf_sb")
            k2_sb = big.tile([D, N], bf16, tag="k", name="k2_sb")
            v2_sb = big.tile([P, NT, C], bf16, tag="v", name="v2_sb")

            def ag_phase(h):
                hs = slice(h * IC, (h + 1) * IC)
                ag_in = dram.tile(
                    [C, IC], bf16, tag=f"ag_in{h}", name=f"ag_in{h}"
                )
                ag_out = dram.tile(
                    [RSH, C, IC], bf16, tag=f"ag_out{h}", name=f"ag_out{h}"
                )
                for ct in range(2):
                    nc.sync.dma_start(
                        out=ag_in[ct * P : (ct + 1) * P, :],
                        in_=xp16_sb[:, ct, hs],
                    )
                nc.gpsimd.collective_compute(
                    "AllGather",
                    OP.bypass,
                    replica_groups=groups,
                    ins=[ag_in[:].opt()],
                    outs=[ag_out[:].opt()],
                )
                for ct in range(2):
                    for r in range(RSH):
                        nc.sync.dma_start(
                            out=xpf_sb[
                                :, ct, r * R + h * IC : r * R + (h + 1) * IC
                            ],
                            in_=ag_out[r, ct * P : (ct + 1) * P, :],
                        )

            def fb_convs(jc):
                """feature_b masking (GpSimd) -> K2 and V2^T convs for one
                512-column chunk of gathered x'."""
                js = slice(jc * IC, (jc + 1) * IC)
                fb0 = fbpool.tile([P, IC], bf16, tag="fb", name="fb0")
                fb1 = fbpool.tile([P, IC], bf16, tag="fb", name="fb1")
                nc.gpsimd.tensor_mul(
                    fb0[:], rmask_sb[:, js], xpf_sb[:, 0, js]
                )
                nc.gpsimd.tensor_mul(
                    fb1[:], rmask_sb[:, js], xpf_sb[:, 1, js]
                )
                ps = psA.tile([D, IC], f32, tag="a", name="k2_ps")
                nc.tensor.matmul(
                    ps[:], wkT2_sb[:, 0, :], fb0[:], start=True, stop=False
                )
                nc.tensor.matmul(
                    ps[:], wkT2_sb[:, 1, :], fb1[:], start=False, stop=True
                )
                nc.vector.tensor_copy(k2_sb[:, js], ps[:])
                for tsub in range(IC // P):
                    t = jc * (IC // P) + tsub
                    ts_ = slice(tsub * P, (tsub + 1) * P)
                    psv = psA.tile([P, C], f32, tag="a", name="v2_ps")
                    nc.tensor.matmul(
                        psv[:], fb0[:, ts_], wvT2_sb[:, 0, :],
                        start=True, stop=False,
                    )
                    nc.tensor.matmul(
                        psv[:], fb1[:, ts_], wvT2_sb[:, 1, :],
                        start=False, stop=True,
                    )
                    nc.scalar.copy(v2_sb[:, t, :], psv[:])

            ag_phase(0)
            for jc in (0, 2, 4, 6):
                fb_convs(jc)
            ag_phase(1)

            # ============== Layer 2: masked cross-attention ==============
            # feature_f chunk + its per-channel stats (sums in stats cols
            # 0-1, sumsqs in 4-5)
            ffsq = misc.tile([P, R], f32, tag="ffsq", name="ffsq")
            for ct in range(2):
                nc.vector.tensor_mul(
                    ff_sb[:, ct, :], maskc_sb[:], xp_sb[:, ct, :]
                )
                nc.vector.tensor_reduce(
                    stats_sb[:, ct : ct + 1], ff_sb[:, ct, :].bitcast(f32),
                    axis=AX.X, op=OP.add,
                )
                nc.vector.scalar_tensor_tensor(
                    ffsq[:], ff_sb[:, ct, :].bitcast(f32), 1.0,
                    ff_sb[:, ct, :].bitcast(f32),
                    op0=OP.mult, op1=OP.mult,
                    accum_out=stats_sb[:, 4 + ct : 5 + ct],
                )

            q2_sb = big.tile([D, R], bf16, tag="q", name="q2_sb")
            conv_qk(wqT2_sb, 8, lambda k, js: ff_sb[:, k, js], R, q2_sb)

            def odd_hook():
                for jc in (1, 3, 5, 7):
                    fb_convs(jc)

            def epilogue2(ich, io, ic, accs, rrep):
                # normalized sw_bg chunk in natural layout; accumulate
                # per-channel sum/sumsq into stats cols 2-3 / 6-7 via VE
                for ct in range(2):
                    onb = onpool.tile([P, ic], f32, tag="on", name="on2")
                    s1 = rcpool.tile([P, 1], f32, tag="s1", name="s1")
                    nc.vector.scalar_tensor_tensor(
                        onb[:], accs[ct][:], 1.0, rrep[:],
                        op0=OP.mult, op1=OP.mult, accum_out=s1[:],
                    )
                    sqb = sqpool.tile([P, ic], f32, tag="sq", name="sq2")
                    s2 = rcpool.tile([P, 1], f32, tag="s2", name="s2")
                    nc.vector.scalar_tensor_tensor(
                        sqb[:], onb[:], 1.0, onb[:],
                        op0=OP.mult, op1=OP.mult, accum_out=s2[:],
                    )
                    if ich == 0:
                        nc.vector.tensor_copy(
                            stats_sb[:, 2 + ct : 3 + ct], s1[:]
                        )
                        nc.vector.tensor_copy(
                            stats_sb[:, 6 + ct : 7 + ct], s2[:]
                        )
                    else:
                        nc.vector.tensor_add(
                            stats_sb[:, 2 + ct : 3 + ct],
                            stats_sb[:, 2 + ct : 3 + ct], s1[:],
                        )
                        nc.vector.tensor_add(
                            stats_sb[:, 6 + ct : 7 + ct],
                            stats_sb[:, 6 + ct : 7 + ct], s2[:],
                        )

            # layer-2 i-chunks (512, 256, 256): the small final chunk
            # shortens the serial den->stats chain ahead of the AllReduce
            attention(
                q2_sb, k2_sb, v2_sb, epilogue2, [IC, IC // 2, IC // 2],
                orders=[TILES_PH0 + TILES_PH1, None, None],
                hooks={(0, len(TILES_PH0)): odd_hook},
            )

            # ================== stats AllReduce + FMM ==================
            ar_in = dram.tile([P, 8], f32, tag="ar_in", name="ar_in")
            ar_out = dram.tile([P, 8], f32, tag="ar_out", name="ar_out")
            nc.sync.dma_start(out=ar_in[:], in_=stats_sb[:])
            nc.gpsimd.collective_compute(
                "AllReduce",
                OP.add,
                replica_groups=groups,
                ins=[ar_in[:].opt()],
                outs=[ar_out[:].opt()],
            )
            rst = misc.tile([P, 8], f32, tag="rst", name="rst")
            nc.sync.dma_start(out=rst[:], in_=ar_out[:])

            # var = (S2 - S1^2/N)/(N-1) + EPS for all four stats at once;
            # ratio = gamma * sqrt(var_sw / var_ff) via scale=gamma^2
            var4 = misc.tile([P, 4], f32, tag="var4", name="var4")
            rat2 = misc.tile([P, 2], f32, tag="rat2", name="rat2")
            ratio = misc.tile([P, 2], f32, tag="ratio", name="ratio")
            nc.vector.scalar_tensor_tensor(
                var4[:], rst[:, 0:4], -1.0 / N, rst[:, 0:4],
                op0=OP.mult, op1=OP.mult,
            )
            nc.vector.tensor_add(var4[:], var4[:], rst[:, 4:8])
            nc.vector.tensor_scalar(
                var4[:], var4[:], 1.0 / (N - 1), EPS, op0=OP.mult, op1=OP.add
            )
            nc.vector.reciprocal(rat2[:], var4[:, 0:2])
            nc.vector.tensor_mul(rat2[:], rat2[:], var4[:, 2:4])
            nc.scalar.activation(
                ratio[:], rat2[:], AF.Sqrt, scale=consts_sb[:, 4:5]
            )

            # out = x' + (gamma * std_bg/std_f) * ff; ct=1 ratio-mul on ACT
            # so the two halves pipeline across engines, DMA per half
            fin0 = finpool.tile([P, R], f32, tag="fin", name="fin0")
            fin1 = finpool.tile([P, R], f32, tag="fin", name="fin1")
            nc.scalar.activation(
                fin1[:], ff_sb[:, 1, :].bitcast(f32), AF.Copy,
                scale=ratio[:, 1:2],
            )
            nc.vector.scalar_tensor_tensor(
                fin0[:], ff_sb[:, 0, :].bitcast(f32),
                ratio[:, 0:1], xp_sb[:, 0, :],
                op0=OP.mult, op1=OP.add,
            )
            nc.sync.dma_start(out=out_d[0:P, :], in_=fin0[:])
            nc.vector.tensor_add(fin1[:], fin1[:], xp_sb[:, 1, :])
            nc.sync.dma_start(out=out_d[P : 2 * P, :], in_=fin1[:])

    nc.compile()
    return nc


def _prep_inputs(x, mask, sa_wq, sa_bq, sa_wk, sa_bk, sa_wv, sa_bv, sa_gamma,
                 wq, bq, wk, bk, wv, bv, gamma):
    """Build the per-core input maps (host-side sharding + weight layout)."""
    x = np.ascontiguousarray(x, dtype=F32)
    mask = np.ascontiguousarray(mask, dtype=F32)

    import ml_dtypes

    BF16 = ml_dtypes.bfloat16
    wqT1 = np.ascontiguousarray(sa_wq.T, dtype=F32)
    wkT1 = np.ascontiguousarray(sa_wk.T.astype(BF16))
    wvT1 = np.ascontiguousarray(sa_wv.T.astype(BF16))
    wqT2 = np.ascontiguousarray(wq.T, dtype=F32)
    wkT2 = np.ascontiguousarray(wk.T.astype(BF16))
    wvT2 = np.ascontiguousarray(wv.T.astype(BF16))

    consts = np.zeros((P, 10), dtype=F32)
    consts[:, 0] = sa_gamma[0]
    consts[:, 1] = gamma[0]
    sgb = (sa_gamma[0] * sa_bv).astype(F32)
    consts[:, 2] = sgb[0:P]
    consts[:, 3] = sgb[P:C]
    consts[:, 4] = gamma[0] * gamma[0]
    consts[0:D, 6] = sa_bq
    consts[0:D, 8] = bq

    in_maps = []
    for g in range(NCORES):
        b, r = g // RSH, g % RSH
        xb = np.ascontiguousarray(x[b].reshape(C, N))
        mb = mask[b].reshape(1, N)
        in_maps.append({
            "xf": np.ascontiguousarray(xb.astype(BF16)),
            "xc": np.ascontiguousarray(xb[:, r * R : (r + 1) * R]),
            "rm": np.ascontiguousarray((1.0 - mb).astype(BF16)),
            "mcrow": np.ascontiguousarray(mb[:, r * R : (r + 1) * R]),
            "wqT1": wqT1, "wkT1": wkT1, "wvT1": wvT1,
            "wqT2": wqT2, "wkT2": wkT2, "wvT2": wvT2,
            "consts": consts,
        })
    return in_maps


def kernel(**inputs):
    from concourse import bass_utils

    if "nc" not in _CACHE:
        _CACHE["nc"] = _build_bass()
    nc = _CACHE["nc"]

    in_maps = _prep_inputs(**inputs)
    res = bass_utils.run_bass_kernel_spmd(
        nc, in_maps, core_ids=list(range(NCORES))
    )
    _CACHE["last_results"] = res

    out = np.empty((B, C, N), dtype=F32)
    for g in range(NCORES):
        b, r = g // RSH, g % RSH
        out[b, :, r * R : (r + 1) * R] = res.results[g]["outc"]
    return out.reshape(B, C, HH, WW)


# revision 14
# speedup vs baseline: 1.2881x; 1.2881x over previous
"""Trainium2 Bass/Tile kernel for nn_FB_FMM (sparse_attention).

Computation (per batch element b, with N = H*W = 4096 tokens, C=256, D=32):
  1. Self-attention:  sa_out = attn(conv(x,sa_wq), conv(x,sa_wk), conv(x,sa_wv))
     x' = sa_gamma * sa_out + x
  2. Masked cross-attention (FB_FMM):
     ff = mask * x'; fb = (1-mask) * x'
     sw_bg = attn(conv(ff,wq), conv(fb,wk), conv(fb,wv))
     out = x' + gamma * ff * (std(sw_bg)/std(ff))    [per-channel std, ddof=1]

Sharding: 8 cores = 2 batch groups x 4-way query-row sharding (1024 rows each).
Each core computes its row-chunk of both attention layers; K/V sides are
computed redundantly per core. Cross-core communication:
  - AllGather of x' chunks within each 4-core batch group (layer-2 K/V need
    the full x'), split into two 512-row phases; layer-2 attention processes
    phase-0 key tiles first so phase 1 hides under compute.
  - AllReduce of per-channel [sum, sumsq] stats for the FMM std ratio.

Layouts: feature maps are channel-major (C on partitions). Scores are computed
transposed (S^T: keys j on partitions, queries i free; logits are small enough
that exp needs no max-subtraction pass). The AV matmul keeps V^T slices
stationary with E^T moving, producing O in natural (c x i) layout. The softmax
denominator is NOT a matmul over E: E tiles are binary-tree-summed on the
Vector engine (bf16) and one ones-matmul per i-chunk reduces the partition
axis; the denominator row is replicated across partitions with a K=1 ones
matmul and reciprocated off the PE critical path. Conv biases: K-side biases
drop out exactly (they add a per-query constant to logits, cancelled by the
row softmax); Q biases are applied on the Scalar engine (Identity+bias).
V-conv PSUM->SBUF copies run on the Scalar engine, K copies on Vector, and
the feature_b masking muls on GpSimd, balancing the three element-wise
engines. V-conv biases fold out mathematically (layer-1 into the residual;
layer-2 via shift invariance of variance). PSUM: 2 conv/den banks + 2 S^T
banks + 4 AV accumulator banks, so consecutive i-chunks overlap without the
PE waiting on epilogues.
"""

import numpy as np

P = 128
B, C, HH, WW = 2, 256, 64, 64
N = HH * WW            # 4096 tokens
D = 32                 # q/k channels
NCORES = 8
RSH = 4                # row shards per batch group
R = N // RSH           # 1024 query rows per core
NT = N // P            # 32 key tiles
IC = 512               # query i-chunk (one PSUM bank of fp32)
EPS = 1e-5
F32 = np.float32

_CACHE = {}

# layer-2 key-tile order: AllGather phase-0 tiles (cols [r*1024, r*1024+512))
# first, phase-1 tiles after
TILES_PH0 = [t for t in range(NT) if (t % 8) < 4]
TILES_PH1 = [t for t in range(NT) if (t % 8) >= 4]


def _build_bass():
    """Build the Bass/Tile program (single SPMD NEFF for all 8 cores)."""
    import concourse.bass as bass
    from concourse import bacc, mybir, tile

    f32 = mybir.dt.float32
    f32r = mybir.dt.float32r
    bf16 = mybir.dt.bfloat16
    AX = mybir.AxisListType
    OP = mybir.AluOpType
    AF = mybir.ActivationFunctionType

    nc = bacc.Bacc(
        "TRN2", target_bir_lowering=False, debug=False, num_devices=NCORES
    )

    # ---------------- I/O ----------------
    xf_d = nc.dram_tensor("xf", [C, N], bf16, kind="ExternalInput")
    xc_d = nc.dram_tensor("xc", [C, R], f32r, kind="ExternalInput")
    rm_d = nc.dram_tensor("rm", [1, N], bf16, kind="ExternalInput")
    mcrow_d = nc.dram_tensor("mcrow", [1, R], f32, kind="ExternalInput")
    wqT1_d = nc.dram_tensor("wqT1", [C, D], f32r, kind="ExternalInput")
    wkT1_d = nc.dram_tensor("wkT1", [C, D], bf16, kind="ExternalInput")
    wvT1_d = nc.dram_tensor("wvT1", [C, C], bf16, kind="ExternalInput")
    wqT2_d = nc.dram_tensor("wqT2", [C, D], f32r, kind="ExternalInput")
    wkT2_d = nc.dram_tensor("wkT2", [C, D], bf16, kind="ExternalInput")
    wvT2_d = nc.dram_tensor("wvT2", [C, C], bf16, kind="ExternalInput")
    # consts columns: 0 sa_gamma, 1 gamma, 2/3 sa_gamma*sa_bv halves,
    # 4 gamma^2, 6 sa_bq, 8 bq (cols 6/8 live on partitions 0..31)
    consts_d = nc.dram_tensor("consts", [P, 10], f32, kind="ExternalInput")
    out_d = nc.dram_tensor("outc", [C, R], f32, kind="ExternalOutput")

    groups = [[0, 1, 2, 3], [4, 5, 6, 7]]

    with tile.TileContext(nc) as tc:
        from contextlib import ExitStack

        ctx = ExitStack()
        with ctx:
            big = ctx.enter_context(tc.tile_pool(name="big", bufs=1))
            epool = ctx.enter_context(tc.tile_pool(name="epool", bufs=4))
            tpool = ctx.enter_context(tc.tile_pool(name="tpool", bufs=7))
            onpool = ctx.enter_context(tc.tile_pool(name="onpool", bufs=3))
            sqpool = ctx.enter_context(tc.tile_pool(name="sqpool", bufs=2))
            fbpool = ctx.enter_context(tc.tile_pool(name="fbpool", bufs=4))
            rcpool = ctx.enter_context(tc.tile_pool(name="rcpool", bufs=4))
            finpool = ctx.enter_context(tc.tile_pool(name="finpool", bufs=2))
            misc = ctx.enter_context(tc.tile_pool(name="misc", bufs=1))
            psA = ctx.enter_context(
                tc.tile_pool(name="psA", bufs=1, space="PSUM")
            )
            psS = ctx.enter_context(
                tc.tile_pool(name="psS", bufs=3, space="PSUM")
            )
            psO = ctx.enter_context(
                tc.tile_pool(name="psO", bufs=4, space="PSUM")
            )
            dram = ctx.enter_context(
                tc.tile_pool(name="dram", bufs=1, space="DRAM")
            )

            # ------------- persistent SBUF tiles -------------
            xc_sb = big.tile([P, 2, R], f32r, tag="xc", name="xc_sb")
            rmask_sb = big.tile([P, N], bf16, tag="rmask", name="rmask_sb")
            maskc_sb = big.tile([P, R], f32, tag="maskc", name="maskc_sb")
            xp_sb = big.tile([P, 2, R], f32, tag="xp", name="xp_sb")
            xp16_sb = big.tile([P, 2, R], bf16, tag="xp16", name="xp16_sb")
            ff_sb = big.tile([P, 2, R], f32r, tag="ff", name="ff_sb")
            wqT1_sb = big.tile([P, 2, D], f32r, tag="wqT1", name="wqT1_sb")
            wkT1_sb = big.tile([P, 2, D], bf16, tag="wkT1", name="wkT1_sb")
            wvT1_sb = big.tile([P, 2, C], bf16, tag="wvT1", name="wvT1_sb")
            wqT2_sb = big.tile([P, 2, D], f32r, tag="wqT2", name="wqT2_sb")
            wkT2_sb = big.tile([P, 2, D], bf16, tag="wkT2", name="wkT2_sb")
            wvT2_sb = big.tile([P, 2, C], bf16, tag="wvT2", name="wvT2_sb")
            consts_sb = big.tile([P, 10], f32, tag="consts", name="consts_sb")
            # ones column (bf16) for the denominator matmul; ones row (f32r)
            # for the K=1 replication matmul
            onesc_sb = big.tile([P, 1], bf16, tag="onesc", name="onesc_sb")
            onesr_sb = big.tile([1, P], f32r, tag="onesr", name="onesr_sb")
            # stats: cols 0-3 = sums (ff0, ff1, sw0, sw1), 4-7 = sumsqs
            stats_sb = misc.tile([P, 8], f32, tag="stats", name="stats_sb")
            xf_sb = big.tile([P, 2, N], bf16, tag="xbig", name="xf_sb")

            # ---- input DMAs, ordered for earliest PE start ----
            # sync queue: Q-conv deps first (consts, wqT1, xc)
            nc.sync.dma_start(out=consts_sb[:], in_=consts_d[:])
            for k in range(2):
                cs = slice(k * P, (k + 1) * P)
                nc.sync.dma_start(out=wqT1_sb[:, k, :], in_=wqT1_d[cs, :])
            for k in range(2):
                cs = slice(k * P, (k + 1) * P)
                nc.sync.dma_start(out=xc_sb[:, k, :], in_=xc_d[cs, :])
            # gpsimd queue: the two big xf halves (1 MB each)
            for k in range(2):
                nc.gpsimd.dma_start(
                    out=xf_sb[:, k, :], in_=xf_d[k * P : (k + 1) * P, :]
                )
            # scalar queue: K/V weights, then layer-2 weights + masks
            for k in range(2):
                cs = slice(k * P, (k + 1) * P)
                nc.scalar.dma_start(out=wkT1_sb[:, k, :], in_=wkT1_d[cs, :])
                nc.scalar.dma_start(out=wvT1_sb[:, k, :], in_=wvT1_d[cs, :])
            for k in range(2):
                cs = slice(k * P, (k + 1) * P)
                nc.scalar.dma_start(out=wqT2_sb[:, k, :], in_=wqT2_d[cs, :])
                nc.scalar.dma_start(out=wkT2_sb[:, k, :], in_=wkT2_d[cs, :])
                nc.scalar.dma_start(out=wvT2_sb[:, k, :], in_=wvT2_d[cs, :])
            nc.scalar.dma_start(
                out=rmask_sb[:], in_=rm_d[0, :].partition_broadcast(P)
            )
            nc.scalar.dma_start(
                out=maskc_sb[:], in_=mcrow_d[0, :].partition_broadcast(P)
            )
            nc.vector.memset(onesc_sb[:], 1.0)
            nc.vector.memset(onesr_sb[:].bitcast(f32), 1.0)

            # ---- collective warmup: tiny AllGather overlapping the head ----
            warm_in = dram.tile([P, 1], f32, tag="warm_in", name="warm_in")
            warm_out = dram.tile(
                [RSH, P, 1], f32, tag="warm_out", name="warm_out"
            )
            nc.sync.dma_start(out=warm_in[:], in_=consts_sb[:, 0:1])
            nc.gpsimd.collective_compute(
                "AllGather",
                OP.bypass,
                replica_groups=groups,
                ins=[warm_in[:].opt()],
                outs=[warm_out[:].opt()],
            )

            def conv_qk(wT_sb, bias_col, src_of, width, out_sb):
                """out (D x width) = wT.T @ src (+ bias via ACT Identity).
                bias_col=None -> plain DVE copy (K convs: bias cancels in
                row-softmax)."""
                for jc in range(width // IC):
                    js = slice(jc * IC, (jc + 1) * IC)
                    ps = psA.tile([D, IC], f32, tag="a", name="qk_ps")
                    nc.tensor.matmul(
                        ps[:], wT_sb[:, 0, :], src_of(0, js),
                        start=True, stop=False,
                    )
                    nc.tensor.matmul(
                        ps[:], wT_sb[:, 1, :], src_of(1, js),
                        start=False, stop=True,
                    )
                    if bias_col is None:
                        nc.vector.tensor_copy(out_sb[:, js], ps[:])
                    else:
                        nc.scalar.activation(
                            out_sb[:, js], ps[:], AF.Identity,
                            bias=consts_sb[0:D, bias_col : bias_col + 1],
                        )

            def conv_vT(wvT_sb, src_of, v_sb, t):
                """v_sb[:, t, :] = (src^T @ wvT) for key tile t (j on
                partitions, channels free)."""
                ts_ = slice(t * P, (t + 1) * P)
                ps = psO.tile([P, C], f32, tag="o", name="v_ps")
                nc.tensor.matmul(
                    ps[:], src_of(0, ts_), wvT_sb[:, 0, :],
                    start=True, stop=False,
                )
                nc.tensor.matmul(
                    ps[:], src_of(1, ts_), wvT_sb[:, 1, :],
                    start=False, stop=True,
                )
                nc.scalar.copy(v_sb[:, t, :], ps[:])

            def attention(q_sb, k_sb, v_sb, epilogue, chunks, orders=None,
                          hooks=None):
                """Row-chunk attention.  Per i-chunk: S^T = K-tile^T Q
                (j on partitions), E = exp(S^T), then O(c,i) accumulates
                with V^T slices stationary and E moving.  The softmax
                denominator comes from a DVE binary-tree sum of the E tiles
                (bf16) + one ones-matmul; the replicated reciprocal is
                computed off the PE critical path.  chunks gives the
                i-chunk widths; orders[ich] the key-tile processing order;
                hooks[(ich, pos)] emits extra work (e.g. convs gated on an
                AllGather) before the s_exp of loop position pos."""
                LOOKAHEAD = 2
                base = 0
                for ich, ic in enumerate(chunks):
                    order = (orders[ich] if orders else None) or list(
                        range(NT)
                    )
                    hk = hooks or {}
                    is_ = slice(base, base + ic)
                    accs = [
                        psO.tile([P, ic], f32, tag="o", name="acc")
                        for _ in range(2)
                    ]
                    es = {}
                    # binary-counter partial sums of E tiles (bf16)
                    partials = [None] * 6

                    def s_exp(pos):
                        t = order[pos]
                        sps = psS.tile([P, ic], f32, tag="s", name="s_ps")
                        nc.tensor.matmul(
                            sps[:],
                            k_sb[:, t * P : (t + 1) * P],
                            q_sb[:, is_],
                            start=True, stop=True,
                        )
                        e_sb = epool.tile([P, ic], bf16, tag="e", name="e_sb")
                        nc.scalar.activation(e_sb[:], sps[:], AF.Exp)
                        es[pos] = e_sb
                        # fold into the tree (DVE, bf16)
                        carry, lev = e_sb, 0
                        while partials[lev] is not None:
                            nxt = tpool.tile(
                                [P, ic], bf16, tag="tp", name="tp"
                            )
                            nc.vector.tensor_add(
                                nxt[:], partials[lev][:], carry[:]
                            )
                            partials[lev] = None
                            carry, lev = nxt, lev + 1
                        partials[lev] = carry

                    for pos in range(LOOKAHEAD):
                        if (ich, pos) in hk:
                            hk[(ich, pos)]()
                        s_exp(pos)
                    for pos in range(NT):
                        if pos + LOOKAHEAD < NT:
                            if (ich, pos + LOOKAHEAD) in hk:
                                hk[(ich, pos + LOOKAHEAD)]()
                            s_exp(pos + LOOKAHEAD)
                        t = order[pos]
                        e_sb = es.pop(pos)
                        for ct in range(2):
                            nc.tensor.matmul(
                                accs[ct][:],
                                v_sb[:, t, ct * P : (ct + 1) * P],
                                e_sb[:],
                                start=(pos == 0), stop=(pos == NT - 1),
                            )
                    esum = partials[5]  # 32 tiles -> exactly level 5
                    den = psA.tile([1, ic], f32, tag="a", name="den")
                    nc.tensor.matmul(
                        den[:], onesc_sb[:], esum[:], start=True, stop=True
                    )
                    denr = rcpool.tile([1, ic], f32r, tag="rc", name="denr")
                    nc.vector.tensor_copy(denr[:], den[:])
                    drep_ps = psA.tile([P, ic], f32, tag="a", name="drep_ps")
                    nc.tensor.matmul(
                        drep_ps[:], onesr_sb[:], denr[:],
                        start=True, stop=True,
                    )
                    rrep = onpool.tile([P, ic], f32, tag="rr", name="rrep")
                    nc.vector.reciprocal_approx_fast(rrep[:], drep_ps[:])
                    epilogue(ich, is_, ic, accs, rrep)
                    base += ic

            # ================= Layer 1: self-attention =================
            q1_sb = big.tile([D, R], bf16, tag="q", name="q1_sb")
            k1_sb = big.tile([D, N], bf16, tag="k", name="k1_sb")
            v1_sb = big.tile([P, NT, C], bf16, tag="v", name="v1_sb")

            conv_qk(wqT1_sb, 6, lambda k, js: xc_sb[:, k, js], R, q1_sb)
            conv_qk(wkT1_sb, None, lambda k, js: xf_sb[:, k, js], N, k1_sb)
            for t in range(NT):
                conv_vT(wvT1_sb, lambda k, ts_: xf_sb[:, k, ts_], v1_sb, t)

            def epilogue1(ich, io, ic, accs, rrep):
                for ct in range(2):
                    # x' = sa_gamma * (O/den) + sa_gamma*bv + x, fused as
                    # ((O * sa_gamma) * rrep), then ((t + sgb) + x)
                    nc.vector.scalar_tensor_tensor(
                        xp_sb[:, ct, io], accs[ct][:],
                        consts_sb[:, 0:1], rrep[:],
                        op0=OP.mult, op1=OP.mult,
                    )
                    nc.vector.scalar_tensor_tensor(
                        xp_sb[:, ct, io], xp_sb[:, ct, io],
                        consts_sb[:, 2 + ct : 3 + ct],
                        xc_sb[:, ct, io].bitcast(f32),
                        op0=OP.add, op1=OP.add,
                    )
                    nc.vector.tensor_copy(
                        xp16_sb[:, ct, io], xp_sb[:, ct, io]
                    )

            attention(q1_sb, k1_sb, v1_sb, epilogue1, [IC, IC])

            # ====== AllGather x' within each batch group (2 phases) ======
            # Phase h gathers x' columns [h*512, (h+1)*512) of every rank;
            # phase 0 overlaps the second layer-1 attention i-chunk.
            xpf_sb = big.tile([P, 2, N], bf16, tag="xpf", name="xpf_sb")
            k2_sb = big.tile([D, N], bf16, tag="k", name="k2_sb")
            v2_sb = big.tile([P, NT, C], bf16, tag="v", name="v2_sb")

            def ag_phase(h):
                hs = slice(h * IC, (h + 1) * IC)
                ag_in = dram.tile(
                    [C, IC], bf16, tag=f"ag_in{h}", name=f"ag_in{h}"
                )
                ag_out = dram.tile(
                    [RSH, C, IC], bf16, tag=f"ag_out{h}", name=f"ag_out{h}"
                )
                for ct in range(2):
                    nc.sync.dma_start(
                        out=ag_in[ct * P : (ct + 1) * P, :],
                        in_=xp16_sb[:, ct, hs],
                    )
                nc.gpsimd.collective_compute(
                    "AllGather",
                    OP.bypass,
                    replica_groups=groups,
                    ins=[ag_in[:].opt()],
                    outs=[ag_out[:].opt()],
                )
                for ct in range(2):
                    for r in range(RSH):
                        nc.sync.dma_start(
                            out=xpf_sb[
                                :, ct, r * R + h * IC : r * R + (h + 1) * IC
                            ],
                            in_=ag_out[r, ct * P : (ct + 1) * P, :],
                        )

            def fb_convs(jc):
                """feature_b masking (GpSimd) -> K2 and V2^T convs for one
                512-column chunk of gathered x'."""
                js = slice(jc * IC, (jc + 1) * IC)
                fb0 = fbpool.tile([P, IC], bf16, tag="fb", name="fb0")
                fb1 = fbpool.tile([P, IC], bf16, tag="fb", name="fb1")
                nc.vector.tensor_mul(
                    fb0[:], rmask_sb[:, js], xpf_sb[:, 0, js]
                )
                nc.vector.tensor_mul(
                    fb1[:], rmask_sb[:, js], xpf_sb[:, 1, js]
                )
                ps = psA.tile([D, IC], f32, tag="a", name="k2_ps")
                nc.tensor.matmul(
                    ps[:], wkT2_sb[:, 0, :], fb0[:], start=True, stop=False
                )
                nc.tensor.matmul(
                    ps[:], wkT2_sb[:, 1, :], fb1[:], start=False, stop=True
                )
                nc.vector.tensor_copy(k2_sb[:, js], ps[:])
                for tsub in range(IC // P):
                    t = jc * (IC // P) + tsub
                    ts_ = slice(tsub * P, (tsub + 1) * P)
                    psv = psO.tile([P, C], f32, tag="o", name="v2_ps")
                    nc.tensor.matmul(
                        psv[:], fb0[:, ts_], wvT2_sb[:, 0, :],
                        start=True, stop=False,
                    )
                    nc.tensor.matmul(
                        psv[:], fb1[:, ts_], wvT2_sb[:, 1, :],
                        start=False, stop=True,
                    )
                    nc.scalar.copy(v2_sb[:, t, :], psv[:])

            ag_phase(0)
            for jc in (0, 2, 4, 6):
                fb_convs(jc)
            ag_phase(1)

            # ============== Layer 2: masked cross-attention ==============
            # feature_f chunk + its per-channel stats (sums in stats cols
            # 0-1, sumsqs in 4-5)
            ffsq = misc.tile([P, R], f32, tag="ffsq", name="ffsq")
            for ct in range(2):
                nc.vector.tensor_mul(
                    ff_sb[:, ct, :], maskc_sb[:], xp_sb[:, ct, :]
                )
                nc.vector.tensor_reduce(
                    stats_sb[:, ct : ct + 1], ff_sb[:, ct, :].bitcast(f32),
                    axis=AX.X, op=OP.add,
                )
                nc.vector.scalar_tensor_tensor(
                    ffsq[:], ff_sb[:, ct, :].bitcast(f32), 1.0,
                    ff_sb[:, ct, :].bitcast(f32),
                    op0=OP.mult, op1=OP.mult,
                    accum_out=stats_sb[:, 4 + ct : 5 + ct],
                )

            q2_sb = big.tile([D, R], bf16, tag="q", name="q2_sb")
            conv_qk(wqT2_sb, 8, lambda k, js: ff_sb[:, k, js], R, q2_sb)

            def odd_hook():
                for jc in (1, 3, 5, 7):
                    fb_convs(jc)

            def epilogue2(ich, io, ic, accs, rrep):
                # normalized sw_bg chunk in natural layout; accumulate
                # per-channel sum/sumsq into stats cols 2-3 / 6-7 via VE
                for ct in range(2):
                    onb = onpool.tile([P, ic], f32, tag="on", name="on2")
                    s1 = rcpool.tile([P, 1], f32, tag="s1", name="s1")
                    nc.vector.scalar_tensor_tensor(
                        onb[:], accs[ct][:], 1.0, rrep[:],
                        op0=OP.mult, op1=OP.mult, accum_out=s1[:],
                    )
                    sqb = sqpool.tile([P, ic], f32, tag="sq", name="sq2")
                    s2 = rcpool.tile([P, 1], f32, tag="s2", name="s2")
                    nc.vector.scalar_tensor_tensor(
                        sqb[:], onb[:], 1.0, onb[:],
                        op0=OP.mult, op1=OP.mult, accum_out=s2[:],
                    )
                    if ich == 0:
                        nc.vector.tensor_copy(
                            stats_sb[:, 2 + ct : 3 + ct], s1[:]
                        )
                        nc.vector.tensor_copy(
                            stats_sb[:, 6 + ct : 7 + ct], s2[:]
                        )
                    else:
                        nc.vector.tensor_add(
                            stats_sb[:, 2 + ct : 3 + ct],
                            stats_sb[:, 2 + ct : 3 + ct], s1[:],
                        )
                        nc.vector.tensor_add(
                            stats_sb[:, 6 + ct : 7 + ct],
                            stats_sb[:, 6 + ct : 7 + ct], s2[:],
                        )

            attention(
                q2_sb, k2_sb, v2_sb, epilogue2, [IC, IC],
                orders=[TILES_PH0 + TILES_PH1, None],
                hooks={(0, len(TILES_PH0)): odd_hook},
            )

            # ================== stats AllReduce + FMM ==================
            ar_in = dram.tile([P, 8], f32, tag="ar_in", name="ar_in")
            ar_out = dram.tile([P, 8], f32, tag="ar_out", name="ar_out")
            nc.sync.dma_start(out=ar_in[:], in_=stats_sb[:])
            nc.gpsimd.collective_compute(
                "AllReduce",
                OP.add,
                replica_groups=groups,
                ins=[ar_in[:].opt()],
                outs=[ar_out[:].opt()],
            )
            rst = misc.tile([P, 8], f32, tag="rst", name="rst")
            nc.sync.dma_start(out=rst[:], in_=ar_out[:])

            # var = (S2 - S1^2/N)/(N-1) + EPS for all four stats at once;
            # ratio = gamma * sqrt(var_sw / var_ff) via scale=gamma^2
            var4 = misc.tile([P, 4], f32, tag="var4", name="var4")
            rat2 = misc.tile([P, 2], f32, tag="rat2", name="rat2")
            ratio = misc.tile([P, 2], f32, tag="ratio", name="ratio")
            nc.vector.scalar_tensor_tensor(
                var4[:], rst[:, 0:4], -1.0 / N, rst[:, 0:4],
                op0=OP.mult, op1=OP.mult,
            )
            nc.vector.tensor_add(var4[:], var4[:], rst[:, 4:8])
            nc.vector.tensor_scalar(
                var4[:], var4[:], 1.0 / (N - 1), EPS, op0=OP.mult, op1=OP.add
            )
            nc.vector.reciprocal(rat2[:], var4[:, 0:2])
            nc.vector.tensor_mul(rat2[:], rat2[:], var4[:, 2:4])
            nc.scalar.activation(
                ratio[:], rat2[:], AF.Sqrt, scale=consts_sb[:, 4:5]
            )

            # out = x' + (gamma * std_bg/std_f) * ff; ct=1 ratio-mul on ACT
            # so the two halves pipeline across engines, DMA per half
            fin0 = finpool.tile([P, R], f32, tag="fin", name="fin0")
            fin1 = finpool.tile([P, R], f32, tag="fin", name="fin1")
            nc.scalar.activation(
                fin1[:], ff_sb[:, 1, :].bitcast(f32), AF.Copy,
                scale=ratio[:, 1:2],
            )
            nc.vector.scalar_tensor_tensor(
                fin0[:], ff_sb[:, 0, :].bitcast(f32),
                ratio[:, 0:1], xp_sb[:, 0, :],
                op0=OP.mult, op1=OP.add,
            )
            nc.sync.dma_start(out=out_d[0:P, :], in_=fin0[:])
            nc.vector.tensor_add(fin1[:], fin1[:], xp_sb[:, 1, :])
            nc.sync.dma_start(out=out_d[P : 2 * P, :], in_=fin1[:])

    nc.compile()
    return nc


def _prep_inputs(x, mask, sa_wq, sa_bq, sa_wk, sa_bk, sa_wv, sa_bv, sa_gamma,
                 wq, bq, wk, bk, wv, bv, gamma):
    """Build the per-core input maps (host-side sharding + weight layout)."""
    x = np.ascontiguousarray(x, dtype=F32)
    mask = np.ascontiguousarray(mask, dtype=F32)

    import ml_dtypes

    BF16 = ml_dtypes.bfloat16
    wqT1 = np.ascontiguousarray(sa_wq.T, dtype=F32)
    wkT1 = np.ascontiguousarray(sa_wk.T.astype(BF16))
    wvT1 = np.ascontiguousarray(sa_wv.T.astype(BF16))
    wqT2 = np.ascontiguousarray(wq.T, dtype=F32)
    wkT2 = np.ascontiguousarray(wk.T.astype(BF16))
    wvT2 = np.ascontiguousarray(wv.T.astype(BF16))

    consts = np.zeros((P, 10), dtype=F32)
    consts[:, 0] = sa_gamma[0]
    consts[:, 1] = gamma[0]
    sgb = (sa_gamma[0] * sa_bv).astype(F32)
    consts[:, 2] = sgb[0:P]
    consts[:, 3] = sgb[P:C]
    consts[:, 4] = gamma[0] * gamma[0]
    consts[0:D, 6] = sa_bq
    consts[0:D, 8] = bq

    in_maps = []
    for g in range(NCORES):
        b, r = g // RSH, g % RSH
        xb = np.ascontiguousarray(x[b].reshape(C, N))
        mb = mask[b].reshape(1, N)
        in_maps.append({
            "xf": np.ascontiguousarray(xb.astype(BF16)),
            "xc": np.ascontiguousarray(xb[:, r * R : (r + 1) * R]),
            "rm": np.ascontiguousarray((1.0 - mb).astype(BF16)),
            "mcrow": np.ascontiguousarray(mb[:, r * R : (r + 1) * R]),
            "wqT1": wqT1, "wkT1": wkT1, "wvT1": wvT1,
            "wqT2": wqT2, "wkT2": wkT2, "wvT2": wvT2,
            "consts": consts,
        })
    return in_maps


def kernel(**inputs):
    from concourse import bass_utils

    if "nc" not in _CACHE:
        _CACHE["nc"] = _build_bass()
    nc = _CACHE["nc"]

    in_maps = _prep_inputs(**inputs)
    res = bass_utils.run_bass_kernel_spmd(
        nc, in_maps, core_ids=list(range(NCORES))
    )
    _CACHE["last_results"] = res

    out = np.empty((B, C, N), dtype=F32)
    for g in range(NCORES):
        b, r = g // RSH, g % RSH
        out[b, :, r * R : (r + 1) * R] = res.results[g]["outc"]
    return out.reshape(B, C, HH, WW)


# revision 18
# speedup vs baseline: 1.4250x; 1.1063x over previous
"""Trainium2 Bass/Tile kernel for nn_FB_FMM (sparse_attention).

Computation (per batch element b, with N = H*W = 4096 tokens, C=256, D=32):
  1. Self-attention:  sa_out = attn(conv(x,sa_wq), conv(x,sa_wk), conv(x,sa_wv))
     x' = sa_gamma * sa_out + x
  2. Masked cross-attention (FB_FMM):
     ff = mask * x'; fb = (1-mask) * x'
     sw_bg = attn(conv(ff,wq), conv(fb,wk), conv(fb,wv))
     out = x' + gamma * ff * (std(sw_bg)/std(ff))    [per-channel std, ddof=1]

Sharding: 8 cores = 2 batch groups x 4-way query-row sharding (1024 rows each).
Each core computes its row-chunk of both attention layers; K/V sides are
computed redundantly per core. Cross-core communication:
  - AllGather of x' chunks within each 4-core batch group (layer-2 K/V need
    the full x'), split into two 512-row phases; layer-2 attention processes
    phase-0 key tiles first so phase 1 hides under compute.
  - AllReduce of per-channel [sum, sumsq] stats for the FMM std ratio.

Layouts: feature maps are channel-major (C on partitions). Scores are computed
transposed (S^T: keys j on partitions, queries i free; logits are small enough
that exp needs no max-subtraction pass). The AV matmul keeps V^T slices
stationary with E^T moving, producing O in natural (c x i) layout. The softmax
denominator is NOT a matmul over E: E tiles are binary-tree-summed on the
Vector engine (bf16) and one ones-matmul per i-chunk reduces the partition
axis; the denominator row is replicated across partitions with a K=1 ones
matmul and reciprocated off the PE critical path. Conv biases: K-side biases
drop out exactly (they add a per-query constant to logits, cancelled by the
row softmax); Q biases are applied on the Scalar engine (Identity+bias).
V-conv PSUM->SBUF copies run on the Scalar engine, K copies on Vector, and
the feature_b masking muls on GpSimd, balancing the three element-wise
engines. V-conv biases fold out mathematically (layer-1 into the residual;
layer-2 via shift invariance of variance). PSUM: 2 conv/den banks + 2 S^T
banks + 4 AV accumulator banks, so consecutive i-chunks overlap without the
PE waiting on epilogues.
"""

import numpy as np

P = 128
B, C, HH, WW = 2, 256, 64, 64
N = HH * WW            # 4096 tokens
D = 32                 # q/k channels
NCORES = 8
RSH = 4                # row shards per batch group
R = N // RSH           # 1024 query rows per core
NT = N // P            # 32 key tiles
IC = 512               # query i-chunk (one PSUM bank of fp32)
EPS = 1e-5
F32 = np.float32

_CACHE = {}

# layer-2 key-tile order: AllGather phase-0 tiles (cols [r*1024, r*1024+512))
# first, phase-1 tiles after
TILES_PH0 = [t for t in range(NT) if (t % 8) < 4]
TILES_PH1 = [t for t in range(NT) if (t % 8) >= 4]


def _build_bass():
    """Build the Bass/Tile program (single SPMD NEFF for all 8 cores)."""
    import concourse.bass as bass
    from concourse import bacc, mybir, tile

    f32 = mybir.dt.float32
    f32r = mybir.dt.float32r
    bf16 = mybir.dt.bfloat16
    AX = mybir.AxisListType
    OP = mybir.AluOpType
    AF = mybir.ActivationFunctionType

    nc = bacc.Bacc(
        "TRN2", target_bir_lowering=False, debug=False, num_devices=NCORES
    )

    # ---------------- I/O ----------------
    xf_d = nc.dram_tensor("xf", [C, N], bf16, kind="ExternalInput")
    xc_d = nc.dram_tensor("xc", [C, R], f32r, kind="ExternalInput")
    rm_d = nc.dram_tensor("rm", [1, N], bf16, kind="ExternalInput")
    mcrow_d = nc.dram_tensor("mcrow", [1, R], f32, kind="ExternalInput")
    wqT1_d = nc.dram_tensor("wqT1", [C, D], f32r, kind="ExternalInput")
    wkT1_d = nc.dram_tensor("wkT1", [C, D], bf16, kind="ExternalInput")
    wvT1_d = nc.dram_tensor("wvT1", [C, C], bf16, kind="ExternalInput")
    wqT2_d = nc.dram_tensor("wqT2", [C, D], f32r, kind="ExternalInput")
    wkT2_d = nc.dram_tensor("wkT2", [C, D], bf16, kind="ExternalInput")
    wvT2_d = nc.dram_tensor("wvT2", [C, C], bf16, kind="ExternalInput")
    # consts columns: 0 sa_gamma, 1 gamma, 2/3 sa_gamma*sa_bv halves,
    # 4 gamma^2, 6 sa_bq, 8 bq (cols 6/8 live on partitions 0..31)
    consts_d = nc.dram_tensor("consts", [P, 10], f32, kind="ExternalInput")
    out_d = nc.dram_tensor("outc", [C, R], f32, kind="ExternalOutput")

    groups = [[0, 1, 2, 3], [4, 5, 6, 7]]

    with tile.TileContext(nc) as tc:
        from contextlib import ExitStack

        ctx = ExitStack()
        with ctx:
            big = ctx.enter_context(tc.tile_pool(name="big", bufs=1))
            epool = ctx.enter_context(tc.tile_pool(name="epool", bufs=4))
            tpool = ctx.enter_context(tc.tile_pool(name="tpool", bufs=7))
            onpool = ctx.enter_context(tc.tile_pool(name="onpool", bufs=3))
            sqpool = ctx.enter_context(tc.tile_pool(name="sqpool", bufs=2))
            fbpool = ctx.enter_context(tc.tile_pool(name="fbpool", bufs=4))
            rcpool = ctx.enter_context(tc.tile_pool(name="rcpool", bufs=4))
            finpool = ctx.enter_context(tc.tile_pool(name="finpool", bufs=2))
            misc = ctx.enter_context(tc.tile_pool(name="misc", bufs=1))
            psA = ctx.enter_context(
                tc.tile_pool(name="psA", bufs=1, space="PSUM")
            )
            psS = ctx.enter_context(
                tc.tile_pool(name="psS", bufs=2, space="PSUM")
            )
            psO = ctx.enter_context(
                tc.tile_pool(name="psO", bufs=3, space="PSUM")
            )
            dram = ctx.enter_context(
                tc.tile_pool(name="dram", bufs=1, space="DRAM")
            )

            # ------------- persistent SBUF tiles -------------
            xc_sb = big.tile([P, 2, R], f32r, tag="xc", name="xc_sb")
            rmask_sb = big.tile([P, N], bf16, tag="rmask", name="rmask_sb")
            maskc_sb = big.tile([P, R], f32, tag="maskc", name="maskc_sb")
            xp_sb = big.tile([P, 2, R], f32, tag="xp", name="xp_sb")
            xp16_sb = big.tile([P, 2, R], bf16, tag="xp16", name="xp16_sb")
            ff_sb = big.tile([P, 2, R], f32r, tag="ff", name="ff_sb")
            wqT1_sb = big.tile([P, 2, D], f32r, tag="wqT1", name="wqT1_sb")
            wkT1_sb = big.tile([P, 2, D], bf16, tag="wkT1", name="wkT1_sb")
            wvT1_sb = big.tile([P, 2, C], bf16, tag="wvT1", name="wvT1_sb")
            wqT2_sb = big.tile([P, 2, D], f32r, tag="wqT2", name="wqT2_sb")
            wkT2_sb = big.tile([P, 2, D], bf16, tag="wkT2", name="wkT2_sb")
            wvT2_sb = big.tile([P, 2, C], bf16, tag="wvT2", name="wvT2_sb")
            consts_sb = big.tile([P, 10], f32, tag="consts", name="consts_sb")
            # ones column (bf16) for the denominator matmul; ones row (f32r)
            # for the K=1 replication matmul
            onesc_sb = big.tile([P, 1], bf16, tag="onesc", name="onesc_sb")
            onesr_sb = big.tile([1, P], f32r, tag="onesr", name="onesr_sb")
            # stats: cols 0-3 = sums (ff0, ff1, sw0, sw1), 4-7 = sumsqs
            stats_sb = misc.tile([P, 8], f32, tag="stats", name="stats_sb")
            xf_sb = big.tile([P, 2, N], bf16, tag="xbig", name="xf_sb")

            # ---- input DMAs, ordered for earliest PE start ----
            # sync queue: Q-conv deps first (consts, wqT1, xc)
            nc.sync.dma_start(out=consts_sb[:], in_=consts_d[:])
            for k in range(2):
                cs = slice(k * P, (k + 1) * P)
                nc.sync.dma_start(out=wqT1_sb[:, k, :], in_=wqT1_d[cs, :])
            for k in range(2):
                cs = slice(k * P, (k + 1) * P)
                nc.sync.dma_start(out=xc_sb[:, k, :], in_=xc_d[cs, :])
            # gpsimd queue: the two big xf halves (1 MB each)
            for k in range(2):
                nc.gpsimd.dma_start(
                    out=xf_sb[:, k, :], in_=xf_d[k * P : (k + 1) * P, :]
                )
            # scalar queue: K/V weights, then layer-2 weights + masks
            for k in range(2):
                cs = slice(k * P, (k + 1) * P)
                nc.scalar.dma_start(out=wkT1_sb[:, k, :], in_=wkT1_d[cs, :])
                nc.scalar.dma_start(out=wvT1_sb[:, k, :], in_=wvT1_d[cs, :])
            for k in range(2):
                cs = slice(k * P, (k + 1) * P)
                nc.scalar.dma_start(out=wqT2_sb[:, k, :], in_=wqT2_d[cs, :])
                nc.scalar.dma_start(out=wkT2_sb[:, k, :], in_=wkT2_d[cs, :])
                nc.scalar.dma_start(out=wvT2_sb[:, k, :], in_=wvT2_d[cs, :])
            nc.scalar.dma_start(
                out=rmask_sb[:], in_=rm_d[0, :].partition_broadcast(P)
            )
            nc.scalar.dma_start(
                out=maskc_sb[:], in_=mcrow_d[0, :].partition_broadcast(P)
            )
            nc.vector.memset(onesc_sb[:], 1.0)
            nc.vector.memset(onesr_sb[:].bitcast(f32), 1.0)

            # ---- collective warmup: tiny AllGather overlapping the head ----
            warm_in = dram.tile([P, 1], f32, tag="warm_in", name="warm_in")
            warm_out = dram.tile(
                [RSH, P, 1], f32, tag="warm_out", name="warm_out"
            )
            nc.sync.dma_start(out=warm_in[:], in_=consts_sb[:, 0:1])
            nc.gpsimd.collective_compute(
                "AllGather",
                OP.bypass,
                replica_groups=groups,
                ins=[warm_in[:].opt()],
                outs=[warm_out[:].opt()],
            )

            def conv_qk(wT_sb, bias_col, src_of, width, out_sb):
                """out (D x width) = wT.T @ src (+ bias via ACT Identity).
                bias_col=None -> plain DVE copy (K convs: bias cancels in
                row-softmax)."""
                for jc in range(width // IC):
                    js = slice(jc * IC, (jc + 1) * IC)
                    ps = psA.tile([D, IC], f32, tag="a", name="qk_ps")
                    nc.tensor.matmul(
                        ps[:], wT_sb[:, 0, :], src_of(0, js),
                        start=True, stop=False,
                    )
                    nc.tensor.matmul(
                        ps[:], wT_sb[:, 1, :], src_of(1, js),
                        start=False, stop=True,
                    )
                    if bias_col is None:
                        nc.vector.tensor_copy(out_sb[:, js], ps[:])
                    else:
                        nc.scalar.activation(
                            out_sb[:, js], ps[:], AF.Identity,
                            bias=consts_sb[0:D, bias_col : bias_col + 1],
                        )

            def conv_vT(wvT_sb, src_of, v_sb, t):
                """v_sb[:, t, :] = (src^T @ wvT) for key tile t (j on
                partitions, channels free).  PSUM drains alternate between
                the Scalar and Vector engines so neither rate-limits."""
                ts_ = slice(t * P, (t + 1) * P)
                ps = psO.tile([P, C], f32, tag="o", name="v_ps")
                nc.tensor.matmul(
                    ps[:], src_of(0, ts_), wvT_sb[:, 0, :],
                    start=True, stop=False,
                )
                nc.tensor.matmul(
                    ps[:], src_of(1, ts_), wvT_sb[:, 1, :],
                    start=False, stop=True,
                )
                if t % 2 == 0:
                    nc.scalar.copy(v_sb[:, t, :], ps[:])
                else:
                    nc.vector.tensor_copy(v_sb[:, t, :], ps[:])

            def attention(q_sb, k_sb, v_sb, epilogue, chunks, orders=None,
                          hooks=None):
                """Row-chunk attention.  Per i-chunk: S^T = K-tile^T Q
                (j on partitions), E = exp(S^T), then O(c,i) accumulates
                with V^T slices stationary and E moving.  The softmax
                denominator comes from a DVE binary-tree sum of the E tiles
                (bf16) + one ones-matmul; the replicated reciprocal is
                computed off the PE critical path.  chunks gives the
                i-chunk widths; orders[ich] the key-tile processing order;
                hooks[(ich, pos)] emits extra work (e.g. convs gated on an
                AllGather) before the s_exp of loop position pos."""
                LOOK = 2  # lookahead in tile PAIRS
                NP = NT // 2
                base = 0
                for ich, ic in enumerate(chunks):
                    order = (orders[ich] if orders else None) or list(
                        range(NT)
                    )
                    hk = hooks or {}
                    is_ = slice(base, base + ic)
                    accs = [
                        psO.tile([P, ic], f32, tag="o", name="acc")
                        for _ in range(2)
                    ]
                    es = {}
                    # binary-counter partial sums of E tiles (bf16);
                    # level 1 entries are pair sums
                    partials = [None] * 6

                    def s_exp_pair(g):
                        """S^T + exp for tile pair (2g, 2g+1): two matmuls
                        into adjacent PSUM banks, ONE exp over both (halves
                        the ACT per-instruction overhead)."""
                        sps = psS.tile([P, 2, ic], f32, tag="s", name="s_ps")
                        for h in range(2):
                            t = order[2 * g + h]
                            nc.tensor.matmul(
                                sps[:, h, :],
                                k_sb[:, t * P : (t + 1) * P],
                                q_sb[:, is_],
                                start=True, stop=True,
                            )
                        e2 = epool.tile(
                            [P, 2, ic], bf16, tag="e", name="e2"
                        )
                        nc.scalar.activation(e2[:], sps[:], AF.Exp)
                        es[g] = e2
                        # pair-sum, then fold into the tree (DVE, bf16)
                        carry = tpool.tile([P, ic], bf16, tag="tp", name="tp")
                        nc.vector.tensor_add(
                            carry[:], e2[:, 0, :], e2[:, 1, :]
                        )
                        lev = 1
                        while partials[lev] is not None:
                            nxt = tpool.tile(
                                [P, ic], bf16, tag="tp", name="tp"
                            )
                            nc.vector.tensor_add(
                                nxt[:], partials[lev][:], carry[:]
                            )
                            partials[lev] = None
                            carry, lev = nxt, lev + 1
                        partials[lev] = carry

                    for g in range(LOOK):
                        if (ich, 2 * g) in hk:
                            hk[(ich, 2 * g)]()
                        s_exp_pair(g)
                    for pos in range(NT):
                        g, h = divmod(pos, 2)
                        if h == 0 and g + LOOK < NP:
                            if (ich, 2 * (g + LOOK)) in hk:
                                hk[(ich, 2 * (g + LOOK))]()
                            s_exp_pair(g + LOOK)
                        t = order[pos]
                        e2 = es[g]
                        for ct in range(2):
                            nc.tensor.matmul(
                                accs[ct][:],
                                v_sb[:, t, ct * P : (ct + 1) * P],
                                e2[:, h, :],
                                start=(pos == 0), stop=(pos == NT - 1),
                            )
                        if h == 1:
                            del es[g]
                    esum = partials[5]  # 32 tiles -> exactly level 5
                    den = psA.tile([1, ic], f32, tag="a", name="den")
                    nc.tensor.matmul(
                        den[:], onesc_sb[:], esum[:], start=True, stop=True
                    )
                    denr = rcpool.tile([1, ic], f32r, tag="rc", name="denr")
                    nc.vector.tensor_copy(denr[:], den[:])
                    drep_ps = psA.tile([P, ic], f32, tag="a", name="drep_ps")
                    nc.tensor.matmul(
                        drep_ps[:], onesr_sb[:], denr[:],
                        start=True, stop=True,
                    )
                    rrep = onpool.tile([P, ic], f32, tag="rr", name="rrep")
                    nc.vector.reciprocal_approx_fast(rrep[:], drep_ps[:])
                    epilogue(ich, is_, ic, accs, rrep)
                    base += ic

            # ================= Layer 1: self-attention =================
            q1_sb = big.tile([D, R], bf16, tag="q", name="q1_sb")
            k1_sb = big.tile([D, N], bf16, tag="k", name="k1_sb")
            v1_sb = big.tile([P, NT, C], bf16, tag="v", name="v1_sb")

            conv_qk(wqT1_sb, 6, lambda k, js: xc_sb[:, k, js], R, q1_sb)
            conv_qk(wkT1_sb, None, lambda k, js: xf_sb[:, k, js], N, k1_sb)
            for t in range(NT):
                conv_vT(wvT1_sb, lambda k, ts_: xf_sb[:, k, ts_], v1_sb, t)

            def epilogue1(ich, io, ic, accs, rrep):
                for ct in range(2):
                    # x' = sa_gamma * (O/den) + sa_gamma*bv + x, fused as
                    # ((O * sa_gamma) * rrep), then ((t + sgb) + x)
                    nc.vector.scalar_tensor_tensor(
                        xp_sb[:, ct, io], accs[ct][:],
                        consts_sb[:, 0:1], rrep[:],
                        op0=OP.mult, op1=OP.mult,
                    )
                    nc.vector.scalar_tensor_tensor(
                        xp_sb[:, ct, io], xp_sb[:, ct, io],
                        consts_sb[:, 2 + ct : 3 + ct],
                        xc_sb[:, ct, io].bitcast(f32),
                        op0=OP.add, op1=OP.add,
                    )
                    nc.vector.tensor_copy(
                        xp16_sb[:, ct, io], xp_sb[:, ct, io]
                    )

            attention(q1_sb, k1_sb, v1_sb, epilogue1, [IC, IC])

            # ====== AllGather x' within each batch group (2 phases) ======
            # Phase h gathers x' columns [h*512, (h+1)*512) of every rank;
            # phase 0 overlaps the second layer-1 attention i-chunk.
            xpf_sb = big.tile([P, 2, N], bf16, tag="xpf", name="xpf_sb")
            k2_sb = big.tile([D, N], bf16, tag="k", name="k2_sb")
            v2_sb = big.tile([P, NT, C], bf16, tag="v", name="v2_sb")

            def ag_phase(h):
                hs = slice(h * IC, (h + 1) * IC)
                ag_in = dram.tile(
                    [C, IC], bf16, tag=f"ag_in{h}", name=f"ag_in{h}"
                )
                ag_out = dram.tile(
                    [RSH, C, IC], bf16, tag=f"ag_out{h}", name=f"ag_out{h}"
                )
                for ct in range(2):
                    nc.sync.dma_start(
                        out=ag_in[ct * P : (ct + 1) * P, :],
                        in_=xp16_sb[:, ct, hs],
                    )
                nc.gpsimd.collective_compute(
                    "AllGather",
                    OP.bypass,
                    replica_groups=groups,
                    ins=[ag_in[:].opt()],
                    outs=[ag_out[:].opt()],
                )
                for ct in range(2):
                    for r in range(RSH):
                        nc.sync.dma_start(
                            out=xpf_sb[
                                :, ct, r * R + h * IC : r * R + (h + 1) * IC
                            ],
                            in_=ag_out[r, ct * P : (ct + 1) * P, :],
                        )

            def fb_convs(jc):
                """feature_b masking (GpSimd) -> K2 and V2^T convs for one
                512-column chunk of gathered x'."""
                js = slice(jc * IC, (jc + 1) * IC)
                fb0 = fbpool.tile([P, IC], bf16, tag="fb", name="fb0")
                fb1 = fbpool.tile([P, IC], bf16, tag="fb", name="fb1")
                nc.vector.tensor_mul(
                    fb0[:], rmask_sb[:, js], xpf_sb[:, 0, js]
                )
                nc.vector.tensor_mul(
                    fb1[:], rmask_sb[:, js], xpf_sb[:, 1, js]
                )
                ps = psA.tile([D, IC], f32, tag="a", name="k2_ps")
                nc.tensor.matmul(
                    ps[:], wkT2_sb[:, 0, :], fb0[:], start=True, stop=False
                )
                nc.tensor.matmul(
                    ps[:], wkT2_sb[:, 1, :], fb1[:], start=False, stop=True
                )
                nc.vector.tensor_copy(k2_sb[:, js], ps[:])
                for tsub in range(IC // P):
                    t = jc * (IC // P) + tsub
                    ts_ = slice(tsub * P, (tsub + 1) * P)
                    psv = psO.tile([P, C], f32, tag="o", name="v2_ps")
                    nc.tensor.matmul(
                        psv[:], fb0[:, ts_], wvT2_sb[:, 0, :],
                        start=True, stop=False,
                    )
                    nc.tensor.matmul(
                        psv[:], fb1[:, ts_], wvT2_sb[:, 1, :],
                        start=False, stop=True,
                    )
                    nc.scalar.copy(v2_sb[:, t, :], psv[:])

            ag_phase(0)
            for jc in (0, 2, 4, 6):
                fb_convs(jc)
            ag_phase(1)

            # ============== Layer 2: masked cross-attention ==============
            # feature_f chunk + its per-channel stats (sums in stats cols
            # 0-1, sumsqs in 4-5)
            ffsq = misc.tile([P, R], f32, tag="ffsq", name="ffsq")
            for ct in range(2):
                nc.vector.tensor_mul(
                    ff_sb[:, ct, :], maskc_sb[:], xp_sb[:, ct, :]
                )
                nc.vector.tensor_reduce(
                    stats_sb[:, ct : ct + 1], ff_sb[:, ct, :].bitcast(f32),
                    axis=AX.X, op=OP.add,
                )
                nc.vector.scalar_tensor_tensor(
                    ffsq[:], ff_sb[:, ct, :].bitcast(f32), 1.0,
                    ff_sb[:, ct, :].bitcast(f32),
                    op0=OP.mult, op1=OP.mult,
                    accum_out=stats_sb[:, 4 + ct : 5 + ct],
                )

            q2_sb = big.tile([D, R], bf16, tag="q", name="q2_sb")
            conv_qk(wqT2_sb, 8, lambda k, js: ff_sb[:, k, js], R, q2_sb)

            def odd_hook():
                for jc in (1, 3, 5, 7):
                    fb_convs(jc)

            def epilogue2(ich, io, ic, accs, rrep):
                # normalized sw_bg chunk in natural layout; accumulate
                # per-channel sum/sumsq into stats cols 2-3 / 6-7 via VE
                for ct in range(2):
                    onb = onpool.tile([P, ic], f32, tag="on", name="on2")
                    s1 = rcpool.tile([P, 1], f32, tag="s1", name="s1")
                    nc.vector.scalar_tensor_tensor(
                        onb[:], accs[ct][:], 1.0, rrep[:],
                        op0=OP.mult, op1=OP.mult, accum_out=s1[:],
                    )
                    sqb = sqpool.tile([P, ic], f32, tag="sq", name="sq2")
                    s2 = rcpool.tile([P, 1], f32, tag="s2", name="s2")
                    nc.vector.scalar_tensor_tensor(
                        sqb[:], onb[:], 1.0, onb[:],
                        op0=OP.mult, op1=OP.mult, accum_out=s2[:],
                    )
                    if ich == 0:
                        nc.vector.tensor_copy(
                            stats_sb[:, 2 + ct : 3 + ct], s1[:]
                        )
                        nc.vector.tensor_copy(
                            stats_sb[:, 6 + ct : 7 + ct], s2[:]
                        )
                    else:
                        nc.vector.tensor_add(
                            stats_sb[:, 2 + ct : 3 + ct],
                            stats_sb[:, 2 + ct : 3 + ct], s1[:],
                        )
                        nc.vector.tensor_add(
                            stats_sb[:, 6 + ct : 7 + ct],
                            stats_sb[:, 6 + ct : 7 + ct], s2[:],
                        )

            attention(
                q2_sb, k2_sb, v2_sb, epilogue2, [IC, IC],
                orders=[TILES_PH0 + TILES_PH1, None],
                hooks={(0, len(TILES_PH0)): odd_hook},
            )

            # ================== stats AllReduce + FMM ==================
            ar_in = dram.tile([P, 8], f32, tag="ar_in", name="ar_in")
            ar_out = dram.tile([P, 8], f32, tag="ar_out", name="ar_out")
            nc.sync.dma_start(out=ar_in[:], in_=stats_sb[:])
            nc.gpsimd.collective_compute(
                "AllReduce",
                OP.add,
                replica_groups=groups,
                ins=[ar_in[:].opt()],
                outs=[ar_out[:].opt()],
            )
            rst = misc.tile([P, 8], f32, tag="rst", name="rst")
            nc.sync.dma_start(out=rst[:], in_=ar_out[:])

            # var = (S2 - S1^2/N)/(N-1) + EPS for all four stats at once;
            # ratio = gamma * sqrt(var_sw / var_ff) via scale=gamma^2
            var4 = misc.tile([P, 4], f32, tag="var4", name="var4")
            rat2 = misc.tile([P, 2], f32, tag="rat2", name="rat2")
            ratio = misc.tile([P, 2], f32, tag="ratio", name="ratio")
            nc.vector.scalar_tensor_tensor(
                var4[:], rst[:, 0:4], -1.0 / N, rst[:, 0:4],
                op0=OP.mult, op1=OP.mult,
            )
            nc.vector.tensor_add(var4[:], var4[:], rst[:, 4:8])
            nc.vector.tensor_scalar(
                var4[:], var4[:], 1.0 / (N - 1), EPS, op0=OP.mult, op1=OP.add
            )
            nc.vector.reciprocal(rat2[:], var4[:, 0:2])
            nc.vector.tensor_mul(rat2[:], rat2[:], var4[:, 2:4])
            nc.scalar.activation(
                ratio[:], rat2[:], AF.Sqrt, scale=consts_sb[:, 4:5]
            )

            # out = x' + (gamma * std_bg/std_f) * ff; ct=1 ratio-mul on ACT
            # so the halves pipeline across engines, DMA per quarter-chunk
            fin0 = finpool.tile([P, R], f32, tag="fin", name="fin0")
            fin1 = finpool.tile([P, R], f32, tag="fin", name="fin1")
            HF = R // 2
            for hf in range(2):
                hs = slice(hf * HF, (hf + 1) * HF)
                nc.scalar.activation(
                    fin1[:, hs], ff_sb[:, 1, hs].bitcast(f32), AF.Copy,
                    scale=ratio[:, 1:2],
                )
                nc.vector.scalar_tensor_tensor(
                    fin0[:, hs], ff_sb[:, 0, hs].bitcast(f32),
                    ratio[:, 0:1], xp_sb[:, 0, hs],
                    op0=OP.mult, op1=OP.add,
                )
                nc.sync.dma_start(out=out_d[0:P, hs], in_=fin0[:, hs])
            for hf in range(2):
                hs = slice(hf * HF, (hf + 1) * HF)
                nc.vector.tensor_add(
                    fin1[:, hs], fin1[:, hs], xp_sb[:, 1, hs]
                )
                nc.sync.dma_start(
                    out=out_d[P : 2 * P, hs], in_=fin1[:, hs]
                )

    nc.compile()
    return nc


def _prep_inputs(x, mask, sa_wq, sa_bq, sa_wk, sa_bk, sa_wv, sa_bv, sa_gamma,
                 wq, bq, wk, bk, wv, bv, gamma):
    """Build the per-core input maps (host-side sharding + weight layout)."""
    x = np.ascontiguousarray(x, dtype=F32)
    mask = np.ascontiguousarray(mask, dtype=F32)

    import ml_dtypes

    BF16 = ml_dtypes.bfloat16
    wqT1 = np.ascontiguousarray(sa_wq.T, dtype=F32)
    wkT1 = np.ascontiguousarray(sa_wk.T.astype(BF16))
    wvT1 = np.ascontiguousarray(sa_wv.T.astype(BF16))
    wqT2 = np.ascontiguousarray(wq.T, dtype=F32)
    wkT2 = np.ascontiguousarray(wk.T.astype(BF16))
    wvT2 = np.ascontiguousarray(wv.T.astype(BF16))

    consts = np.zeros((P, 10), dtype=F32)
    consts[:, 0] = sa_gamma[0]
    consts[:, 1] = gamma[0]
    sgb = (sa_gamma[0] * sa_bv).astype(F32)
    consts[:, 2] = sgb[0:P]
    consts[:, 3] = sgb[P:C]
    consts[:, 4] = gamma[0] * gamma[0]
    consts[0:D, 6] = sa_bq
    consts[0:D, 8] = bq

    in_maps = []
    for g in range(NCORES):
        b, r = g // RSH, g % RSH
        xb = np.ascontiguousarray(x[b].reshape(C, N))
        mb = mask[b].reshape(1, N)
        in_maps.append({
            "xf": np.ascontiguousarray(xb.astype(BF16)),
            "xc": np.ascontiguousarray(xb[:, r * R : (r + 1) * R]),
            "rm": np.ascontiguousarray((1.0 - mb).astype(BF16)),
            "mcrow": np.ascontiguousarray(mb[:, r * R : (r + 1) * R]),
            "wqT1": wqT1, "wkT1": wkT1, "wvT1": wvT1,
            "wqT2": wqT2, "wkT2": wkT2, "wvT2": wvT2,
            "consts": consts,
        })
    return in_maps


def kernel(**inputs):
    from concourse import bass_utils

    if "nc" not in _CACHE:
        _CACHE["nc"] = _build_bass()
    nc = _CACHE["nc"]

    in_maps = _prep_inputs(**inputs)
    res = bass_utils.run_bass_kernel_spmd(
        nc, in_maps, core_ids=list(range(NCORES))
    )
    _CACHE["last_results"] = res

    out = np.empty((B, C, N), dtype=F32)
    for g in range(NCORES):
        b, r = g // RSH, g % RSH
        out[b, :, r * R : (r + 1) * R] = res.results[g]["outc"]
    return out.reshape(B, C, HH, WW)
